# revision 12
# baseline (speedup 1.0000x reference)
"""Trainium2 Bass kernel for nn_Attention_org_45758581571643.

Reference computation (per batch b):
  x = emb[b] viewed as [S=T*N, C] (token-major)
  per head h: Q/K/V = x @ W{q,k,v}[h].T ; scores = Q K^T / sqrt(S)
  InstanceNorm over each [S,S] map, softmax over keys, ctx = probs @ V
  out = mean_h(ctx) @ Wo.T, reshaped to [B, T, C, N]

Sharding: 16 (batch, head) pairs over 8 cores -> core c handles batch c//2,
heads {2*(c%2), 2*(c%2)+1}. Head-mean and the Wo projection are linear, so each
core applies Wo to its own two-head partial sum and the host adds core pairs.

On-device layout is fully transposed: x/Q/K live as [C, S] (channel on
partitions), scores as [t, s] (keys on partitions). Softmax runs over the
partition axis: denominators via ones-matmuls on the PE, stats via
ACT-copy-with-accum + DVE square-reduce + gpsimd partition_all_reduce.
probs @ V then needs no transposes at all. The 1/sqrt(S) score scaling is
skipped -- instance norm is invariant to it. S is zero-padded to 1664 = 13*128;
padded key/value rows are exactly zero so sums and matmuls stay exact, and the
padded rows are excluded from softmax denominators by a K=32 tail matmul.
"""

import numpy as np
from contextlib import ExitStack

B, T, C, N, H = 4, 8, 256, 196, 4
S = T * N          # 1568
SP = 1664          # 13 * 128 (padded key/seq length)
NT = SP // 128     # 13 t-tiles
SCW = 392          # s-chunk width (4 * 392 = 1568)
NSC = S // SCW     # 4
PAD_REAL = S - (NT - 1) * 128  # 32 real rows in the last t-tile
EPS = 1e-5
COUNT = float(S) * float(S)

_CACHE = {}


def _build_nc():
    import concourse.bass as bass
    import concourse.tile as tile
    from concourse import bacc, bass_isa, mybir

    f32 = mybir.dt.float32
    f32r = mybir.dt.float32r
    AF = mybir.ActivationFunctionType
    ALU = mybir.AluOpType

    nc = bacc.Bacc("TRN2", target_bir_lowering=False, debug=False)

    xt_d = nc.dram_tensor("xt", [C, SP], f32r, kind="ExternalInput").ap()
    wq_d = nc.dram_tensor("wq", [2, C, C], f32r, kind="ExternalInput").ap()
    wk_d = nc.dram_tensor("wk", [2, C, C], f32r, kind="ExternalInput").ap()
    wv_d = nc.dram_tensor("wv", [2, C, C], f32r, kind="ExternalInput").ap()
    wo_d = nc.dram_tensor("wo", [C, C], f32r, kind="ExternalInput").ap()
    ot_d = nc.dram_tensor("ot", [C, S], f32, kind="ExternalOutput").ap()

    def r(ap):
        return ap

    def v32(ap):
        return ap.bitcast(f32)

    with tile.TileContext(nc) as tc, ExitStack() as ctx:
        xw = ctx.enter_context(tc.tile_pool(name="xw", bufs=1))
        qk = ctx.enter_context(tc.tile_pool(name="qk", bufs=1))
        vp = ctx.enter_context(tc.tile_pool(name="vp", bufs=1))
        sc = ctx.enter_context(tc.tile_pool(name="sc", bufs=1))
        cx = ctx.enter_context(tc.tile_pool(name="cx", bufs=1))
        sm = ctx.enter_context(tc.tile_pool(name="sm", bufs=4))
        scr = ctx.enter_context(tc.tile_pool(name="scr", bufs=2))
        pmm = ctx.enter_context(tc.tile_pool(name="pmm", bufs=3, space="PSUM"))
        pcx = ctx.enter_context(tc.tile_pool(name="pcx", bufs=2, space="PSUM"))
        pcs = ctx.enter_context(tc.tile_pool(name="pcs", bufs=2, space="PSUM"))

        # ---- load inputs ----
        xt = [xw.tile([128, SP], f32r, tag=f"xt{i}", name=f"xt{i}") for i in range(2)]
        for cti in range(2):
            nc.sync.dma_start(xt[cti][:], xt_d[cti * 128:(cti + 1) * 128, :])
        wsb = {}
        for nm, d in (("wq", wq_d), ("wk", wk_d), ("wv", wv_d)):
            for h in range(2):
                for cti in range(2):
                    t = xw.tile([128, C], f32r, tag=f"{nm}{h}{cti}", name=f"{nm}{h}{cti}")
                    nc.sync.dma_start(t[:], d[h, cti * 128:(cti + 1) * 128, :])
                    wsb[nm, h, cti] = t
        wo = [xw.tile([128, C], f32r, tag=f"wo{cti}", name=f"wo{cti}") for cti in range(2)]
        for cti in range(2):
            nc.sync.dma_start(wo[cti][:], wo_d[cti * 128:(cti + 1) * 128, :])

        fourf = xw.tile([128, 1], f32, tag="fourf")
        nc.vector.memset(fourf, float(H))
        four = xw.tile([128, 1], f32r, tag="four")
        nc.vector.tensor_copy(four[:], fourf[:])
        epsb = xw.tile([128, 1], f32, tag="epsb")
        nc.vector.memset(epsb, EPS)

        ctxs = {}  # dti -> [128, S] scaled ctx^T summed over this core's heads
        for dti in range(2):
            ctxs[dti] = cx.tile([128, S], f32r, tag=f"ctx{dti}", name=f"ctx{dti}", bufs=1)

        for h in range(2):
            # ---- projections: QT/KT [d, s], V [t, d] ----
            qt = [qk.tile([128, S], f32r, tag=f"qt{i}", name=f"qt{i}", bufs=1) for i in range(2)]
            kt = [qk.tile([128, SP], f32r, tag=f"kt{i}", name=f"kt{i}", bufs=1) for i in range(2)]
            for dti in range(2):
                for sci in range(NSC):
                    sl = slice(sci * SCW, (sci + 1) * SCW)
                    ps = pmm.tile([128, SCW], f32, tag="ps", name="ps")
                    for cti in range(2):
                        nc.tensor.matmul(
                            ps[:], r(wsb["wq", h, cti][:, dti * 128:(dti + 1) * 128]),
                            r(xt[cti][:, sl]), start=(cti == 0), stop=(cti == 1))
                    nc.vector.tensor_copy(qt[dti][:, sl], ps[:])
                for kci in range(4):
                    kl = slice(kci * 416, (kci + 1) * 416)
                    ps = pmm.tile([128, 416], f32, tag="ps", name="ps")
                    for cti in range(2):
                        nc.tensor.matmul(
                            ps[:], r(wsb["wk", h, cti][:, dti * 128:(dti + 1) * 128]),
                            r(xt[cti][:, kl]), start=(cti == 0), stop=(cti == 1))
                    nc.vector.tensor_copy(kt[dti][:, kl], ps[:])
            v = [vp.tile([128, C], f32r, tag=f"v{i}", name=f"v{i}", bufs=1) for i in range(NT)]
            for ti in range(NT):
                tsl = slice(ti * 128, (ti + 1) * 128)
                ps = pmm.tile([128, C], f32, tag="ps", name="ps")
                for cti in range(2):
                    nc.tensor.matmul(
                        ps[:], r(xt[cti][:, tsl]), r(wsb["wv", h, cti][:]),
                        start=(cti == 0), stop=(cti == 1))
                nc.vector.tensor_copy(v[ti][:], ps[:])

            # ---- scores^T [t, s] + per-tile stats (bn_stats -> sums) ----
            sums = sm.tile([128, NT], f32, tag="sums", name="sums")
            sumsq = sm.tile([128, NT], f32, tag="sumsq", name="sumsq")
            st = [sc.tile([128, S], f32r, tag=f"st{i}", name=f"st{i}", bufs=1) for i in range(NT)]
            for ti in range(NT):
                tsl = slice(ti * 128, (ti + 1) * 128)
                bst = sm.tile([128, NSC, 6], f32, tag="bst", name="bst", bufs=3)
                for sci in range(NSC):
                    sl = slice(sci * SCW, (sci + 1) * SCW)
                    ps = pmm.tile([128, SCW], f32, tag="ps", name="ps")
                    for cti in range(2):
                        nc.tensor.matmul(
                            ps[:], r(kt[cti][:, tsl]), r(qt[cti][:, sl]),
                            start=(cti == 0), stop=(cti == 1))
                    nc.scalar.activation(out=st[ti][:, sl], in_=ps[:], func=AF.Copy)
                    nc.vector.bn_stats(out=bst[:, sci, :], in_=v32(st[ti][:, sl]))
                mv = sm.tile([128, 2], f32, tag="mv", name="mv", bufs=3)
                nc.vector.bn_aggr(out=mv[:], in_=bst[:])
                # per-row sum = mean * S ; sumsq = (var + mean^2) * S
                nc.vector.tensor_scalar_mul(sums[:, ti:ti + 1], mv[:, 0:1], float(S))
                t2 = sm.tile([128, 1], f32, tag="t2s", name="t2s", bufs=3)
                nc.vector.scalar_tensor_tensor(
                    out=t2[:], in0=mv[:, 0:1], scalar=mv[:, 0:1], in1=mv[:, 1:2],
                    op0=ALU.mult, op1=ALU.add)
                nc.vector.tensor_scalar_mul(sumsq[:, ti:ti + 1], t2[:], float(S))

            # ---- instance-norm scalars (replicated across partitions) ----
            st2 = sm.tile([128, 2], f32, tag="st2", name="st2")
            nc.vector.tensor_reduce(st2[:, 0:1], sums[:], axis=mybir.AxisListType.X,
                                    op=ALU.add)
            nc.vector.tensor_reduce(st2[:, 1:2], sumsq[:], axis=mybir.AxisListType.X,
                                    op=ALU.add)
            red = sm.tile([128, 2], f32, tag="red", name="red")
            nc.gpsimd.partition_all_reduce(red[:], st2[:], channels=128,
                                           reduce_op=bass_isa.ReduceOp.add)
            mean = sm.tile([128, 1], f32, tag="mean", name="mean")
            nc.vector.tensor_scalar_mul(mean[:], red[:, 0:1], 1.0 / COUNT)
            mm2 = sm.tile([128, 1], f32, tag="mm2", name="mm2")
            nc.vector.tensor_mul(mm2[:], mean[:], mean[:])
            var = sm.tile([128, 1], f32, tag="var", name="var")
            nc.vector.scalar_tensor_tensor(
                out=var[:], in0=red[:, 1:2], scalar=1.0 / COUNT, in1=mm2[:],
                op0=ALU.mult, op1=ALU.subtract)
            rstd = sm.tile([128, 1], f32, tag="rstd", name="rstd")
            nc.scalar.activation(out=rstd[:], in_=var[:], func=AF.Sqrt, bias=epsb[:])
            nc.vector.reciprocal(rstd[:], rstd[:])
            nbias = sm.tile([128, 1], f32, tag="nbias", name="nbias")
            nc.vector.scalar_tensor_tensor(
                out=nbias[:], in0=mean[:], scalar=-1.0, in1=rstd[:],
                op0=ALU.mult, op1=ALU.mult)

            # ---- p = exp(rstd * scores + nbias), in place ----
            for ti in range(NT):
                nc.scalar.activation(out=st[ti][:], in_=st[ti][:], func=AF.Exp,
                                     bias=nbias[:], scale=rstd[:])

            # ---- softmax denominators (x H), reciprocal, broadcast ----
            den = sm.tile([1, S], f32, tag="den", name="den", bufs=2)
            for sci in range(NSC):
                sl = slice(sci * SCW, (sci + 1) * SCW)
                pd = pcs.tile([1, SCW], f32, tag="pd", name="pd")
                for ti in range(NT):
                    kk = 128 if ti < NT - 1 else PAD_REAL
                    nc.tensor.matmul(
                        pd[:], r(four[0:kk, :]), r(st[ti][0:kk, sl]),
                        start=(ti == 0), stop=(ti == NT - 1))
                nc.vector.tensor_copy(den[0:1, sl], pd[:])
            nc.vector.reciprocal(den[:], den[:])
            recipb = scr.tile([128, S], f32, tag="recipb", name="recipb", bufs=1)
            nc.gpsimd.partition_broadcast(recipb[:], den[0:1, :])

            # ---- ctx^T[d, s] = V^T p, scaled by 1/(H * denom) ----
            for dti in range(2):
                dsl = slice(dti * 128, (dti + 1) * 128)
                for sci in range(NSC):
                    sl = slice(sci * SCW, (sci + 1) * SCW)
                    ps = pcx.tile([128, SCW], f32, tag="psx", name="psx")
                    for ti in range(NT):
                        nc.tensor.matmul(ps[:], r(v[ti][:, dsl]), r(st[ti][:, sl]),
                                         start=(ti == 0), stop=(ti == NT - 1))
                    if h == 0:
                        nc.vector.tensor_mul(ctxs[dti][:, sl], ps[:], recipb[:, sl])
                    else:
                        t3 = scr.tile([128, SCW], f32, tag="t2", name="t3")
                        nc.vector.tensor_mul(t3[:], ps[:], recipb[:, sl])
                        nc.vector.tensor_add(ctxs[dti][:, sl], ctxs[dti][:, sl], t3[:])

        # ---- output projection: ot[e, s] = Wo^T @ (ctx0 + ctx1) ----
        for eti in range(2):
            esl = slice(eti * 128, (eti + 1) * 128)
            osb = scr.tile([128, S], f32, tag=f"osb{eti}", name=f"osb{eti}", bufs=1)
            for sci in range(NSC):
                sl = slice(sci * SCW, (sci + 1) * SCW)
                ps = pmm.tile([128, SCW], f32, tag="ps", name="ps")
                for cti in range(2):
                    nc.tensor.matmul(ps[:], r(wo[cti][:, esl]),
                                     r(ctxs[cti][:, sl]),
                                     start=(cti == 0), stop=(cti == 1))
                nc.vector.tensor_copy(osb[:, sl], ps[:])
            nc.sync.dma_start(ot_d[esl, :], osb[:])

    nc.finalize()
    return nc


def _get_nc():
    if "nc" not in _CACHE:
        _CACHE["nc"] = _build_nc()
    return _CACHE["nc"]


def make_in_maps(emb, Wq, Wk, Wv, Wo):
    emb = np.ascontiguousarray(emb, dtype=np.float32)
    woT = np.ascontiguousarray(np.asarray(Wo, np.float32).T)
    wqT = np.ascontiguousarray(np.asarray(Wq, np.float32).transpose(0, 2, 1))
    wkT = np.ascontiguousarray(np.asarray(Wk, np.float32).transpose(0, 2, 1))
    wvT = np.ascontiguousarray(np.asarray(Wv, np.float32).transpose(0, 2, 1))
    in_maps = []
    for core in range(8):
        b, g = core // 2, core % 2
        xt = np.zeros((C, SP), np.float32)
        xt[:, :S] = emb[b].transpose(1, 0, 2).reshape(C, S)
        hs = [2 * g, 2 * g + 1]
        in_maps.append({
            "xt": xt,
            "wq": np.ascontiguousarray(wqT[hs]),
            "wk": np.ascontiguousarray(wkT[hs]),
            "wv": np.ascontiguousarray(wvT[hs]),
            "wo": woT,
        })
    return in_maps


def gather_out(results):
    out = np.empty((B, S, C), np.float32)
    for b in range(B):
        out[b] = (results[2 * b]["ot"] + results[2 * b + 1]["ot"]).T
    return out.reshape(B, T, C, N)


def kernel(emb, Wq, Wk, Wv, Wo):
    from concourse.bass_utils import run_bass_kernel_spmd

    nc = _get_nc()
    in_maps = make_in_maps(emb, Wq, Wk, Wv, Wo)
    res = run_bass_kernel_spmd(nc, in_maps, list(range(8)))
    return gather_out(res.results)


# revision 18
# speedup vs baseline: 1264.9241x; 1264.9241x over previous
"""Trainium2 Bass kernel for nn_Attention_org_45758581571643.

Reference computation (per batch b):
  x = emb[b] viewed as [S=T*N, C] (token-major)
  per head h: Q/K/V = x @ W{q,k,v}[h].T ; scores = Q K^T / sqrt(S)
  InstanceNorm over each [S,S] map, softmax over keys, ctx = probs @ V
  out = mean_h(ctx) @ Wo.T, reshaped to [B, T, C, N]

Sharding: 16 (batch, head) pairs over 8 cores -> core c handles batch c//2,
heads {2*(c%2), 2*(c%2)+1}. Head-mean and the Wo projection are linear, so each
core applies Wo to its own two-head partial sum and the host adds core pairs.

On-device layout is fully transposed: x/Q/K live as [C, S] (channel on
partitions), scores as [t, s] (keys on partitions). Softmax runs over the
partition axis: denominators via ones-matmuls on the PE, stats via
ACT-copy-with-accum + DVE square-reduce + gpsimd partition_all_reduce.
probs @ V then needs no transposes at all. The 1/sqrt(S) score scaling is
skipped -- instance norm is invariant to it. S is zero-padded to 1664 = 13*128;
padded key/value rows are exactly zero so sums and matmuls stay exact, and the
padded rows are excluded from softmax denominators by a K=32 tail matmul.
"""

import os

# Recover gracefully if a previous run left a NeuronCore wedged; must be set
# before the runtime initializes.
os.environ.setdefault("NEURON_RT_RESET_CORES", "1")

import numpy as np
from contextlib import ExitStack

B, T, C, N, H = 4, 8, 256, 196, 4
S = T * N          # 1568
SP = 1664          # 13 * 128 (padded key/seq length)
NT = SP // 128     # 13 t-tiles
SCW = 392          # s-chunk width (4 * 392 = 1568)
NSC = S // SCW     # 4
PAD_REAL = S - (NT - 1) * 128  # 32 real rows in the last t-tile
EPS = 1e-5
COUNT = float(S) * float(S)

_CACHE = {}


def _build_nc(reps=1):
    import concourse.bass as bass
    import concourse.tile as tile
    from concourse import bacc, bass_isa, mybir

    f32 = mybir.dt.float32
    f32r = mybir.dt.float32r
    AF = mybir.ActivationFunctionType
    ALU = mybir.AluOpType

    nc = bacc.Bacc("TRN2", target_bir_lowering=False, debug=False)

    xt_d = nc.dram_tensor("xt", [C, SP], f32r, kind="ExternalInput").ap()
    wq_d = nc.dram_tensor("wq", [2, C, C], f32r, kind="ExternalInput").ap()
    wk_d = nc.dram_tensor("wk", [2, C, C], f32r, kind="ExternalInput").ap()
    wv_d = nc.dram_tensor("wv", [2, C, C], f32r, kind="ExternalInput").ap()
    wo_d = nc.dram_tensor("wo", [C, C], f32r, kind="ExternalInput").ap()
    ot_d = nc.dram_tensor("ot", [C, S], f32, kind="ExternalOutput").ap()

    def r(ap):
        return ap

    def v32(ap):
        return ap.bitcast(f32)

    with tile.TileContext(nc) as tc, ExitStack() as ctx:
        xw = ctx.enter_context(tc.tile_pool(name="xw", bufs=1))
        qk = ctx.enter_context(tc.tile_pool(name="qk", bufs=1))
        vp = ctx.enter_context(tc.tile_pool(name="vp", bufs=1))
        sc = ctx.enter_context(tc.tile_pool(name="sc", bufs=1))
        cx = ctx.enter_context(tc.tile_pool(name="cx", bufs=1))
        sm = ctx.enter_context(tc.tile_pool(name="sm", bufs=4))
        scr = ctx.enter_context(tc.tile_pool(name="scr", bufs=2))
        pmm = ctx.enter_context(tc.tile_pool(name="pmm", bufs=3, space="PSUM"))
        pcx = ctx.enter_context(tc.tile_pool(name="pcx", bufs=2, space="PSUM"))
        pcs = ctx.enter_context(tc.tile_pool(name="pcs", bufs=2, space="PSUM"))

        # ---- load inputs ----
        xt = [xw.tile([128, SP], f32r, tag=f"xt{i}", name=f"xt{i}") for i in range(2)]
        for cti in range(2):
            nc.sync.dma_start(xt[cti][:], xt_d[cti * 128:(cti + 1) * 128, :])
        wsb = {}
        for nm, d in (("wq", wq_d), ("wk", wk_d), ("wv", wv_d)):
            for h in range(2):
                for cti in range(2):
                    t = xw.tile([128, C], f32r, tag=f"{nm}{h}{cti}", name=f"{nm}{h}{cti}")
                    nc.sync.dma_start(t[:], d[h, cti * 128:(cti + 1) * 128, :])
                    wsb[nm, h, cti] = t
        wo = [xw.tile([128, C], f32r, tag=f"wo{cti}", name=f"wo{cti}") for cti in range(2)]
        for cti in range(2):
            nc.sync.dma_start(wo[cti][:], wo_d[cti * 128:(cti + 1) * 128, :])

        fourf = xw.tile([128, 1], f32, tag="fourf")
        nc.vector.memset(fourf, float(H))
        four = xw.tile([128, 1], f32r, tag="four")
        nc.vector.tensor_copy(four[:], fourf[:])
        epsb = xw.tile([128, 1], f32, tag="epsb")
        nc.vector.memset(epsb, EPS)

      for _rep in range(reps):
        ctxs = {}  # dti -> [128, S] scaled ctx^T summed over this core's heads
        for dti in range(2):
            ctxs[dti] = cx.tile([128, S], f32r, tag=f"ctx{dti}", name=f"ctx{dti}", bufs=1)

        for h in range(2):
            # ---- projections: QT/KT [d, s], V [t, d] ----
            qt = [qk.tile([128, S], f32r, tag=f"qt{i}", name=f"qt{i}", bufs=1) for i in range(2)]
            kt = [qk.tile([128, SP], f32r, tag=f"kt{i}", name=f"kt{i}", bufs=1) for i in range(2)]
            for dti in range(2):
                for sci in range(NSC):
                    sl = slice(sci * SCW, (sci + 1) * SCW)
                    ps = pmm.tile([128, SCW], f32, tag="ps", name="ps")
                    for cti in range(2):
                        nc.tensor.matmul(
                            ps[:], r(wsb["wq", h, cti][:, dti * 128:(dti + 1) * 128]),
                            r(xt[cti][:, sl]), start=(cti == 0), stop=(cti == 1))
                    nc.vector.tensor_copy(qt[dti][:, sl], ps[:])
                for kci in range(4):
                    kl = slice(kci * 416, (kci + 1) * 416)
                    ps = pmm.tile([128, 416], f32, tag="ps", name="ps")
                    for cti in range(2):
                        nc.tensor.matmul(
                            ps[:], r(wsb["wk", h, cti][:, dti * 128:(dti + 1) * 128]),
                            r(xt[cti][:, kl]), start=(cti == 0), stop=(cti == 1))
                    nc.vector.tensor_copy(kt[dti][:, kl], ps[:])
            v = [vp.tile([128, C], f32r, tag=f"v{i}", name=f"v{i}", bufs=2) for i in range(NT)]
            for ti in range(NT):
                tsl = slice(ti * 128, (ti + 1) * 128)
                ps = pmm.tile([128, C], f32, tag="ps", name="ps")
                for cti in range(2):
                    nc.tensor.matmul(
                        ps[:], r(xt[cti][:, tsl]), r(wsb["wv", h, cti][:]),
                        start=(cti == 0), stop=(cti == 1))
                nc.vector.tensor_copy(v[ti][:], ps[:])

            # ---- scores^T [t, s] + per-tile stats (bn_stats -> sums) ----
            sums = sm.tile([128, NT], f32, tag="sums", name="sums")
            sumsq = sm.tile([128, NT], f32, tag="sumsq", name="sumsq")
            st = [sc.tile([128, S], f32r, tag=f"st{i}", name=f"st{i}", bufs=1) for i in range(NT)]
            for ti in range(NT):
                tsl = slice(ti * 128, (ti + 1) * 128)
                bst = sm.tile([128, NSC, 6], f32, tag="bst", name="bst", bufs=3)
                for sci in range(NSC):
                    sl = slice(sci * SCW, (sci + 1) * SCW)
                    ps = pmm.tile([128, SCW], f32, tag="ps", name="ps")
                    for cti in range(2):
                        nc.tensor.matmul(
                            ps[:], r(kt[cti][:, tsl]), r(qt[cti][:, sl]),
                            start=(cti == 0), stop=(cti == 1))
                    nc.scalar.activation(out=st[ti][:, sl], in_=ps[:], func=AF.Copy)
                    nc.vector.bn_stats(out=bst[:, sci, :], in_=v32(st[ti][:, sl]))
                mv = sm.tile([128, 2], f32, tag="mv", name="mv", bufs=3)
                nc.vector.bn_aggr(out=mv[:], in_=bst[:])
                # per-row sum = mean * S ; sumsq = (var + mean^2) * S
                nc.vector.tensor_scalar_mul(sums[:, ti:ti + 1], mv[:, 0:1], float(S))
                t2 = sm.tile([128, 1], f32, tag="t2s", name="t2s", bufs=3)
                nc.vector.scalar_tensor_tensor(
                    out=t2[:], in0=mv[:, 0:1], scalar=mv[:, 0:1], in1=mv[:, 1:2],
                    op0=ALU.mult, op1=ALU.add)
                nc.vector.tensor_scalar_mul(sumsq[:, ti:ti + 1], t2[:], float(S))

            # ---- instance-norm scalars (replicated across partitions) ----
            st2 = sm.tile([128, 2], f32, tag="st2", name="st2")
            nc.vector.tensor_reduce(st2[:, 0:1], sums[:], axis=mybir.AxisListType.X,
                                    op=ALU.add)
            nc.vector.tensor_reduce(st2[:, 1:2], sumsq[:], axis=mybir.AxisListType.X,
                                    op=ALU.add)
            red = sm.tile([128, 2], f32, tag="red", name="red")
            nc.gpsimd.partition_all_reduce(red[:], st2[:], channels=128,
                                           reduce_op=bass_isa.ReduceOp.add)
            mean = sm.tile([128, 1], f32, tag="mean", name="mean")
            nc.vector.tensor_scalar_mul(mean[:], red[:, 0:1], 1.0 / COUNT)
            mm2 = sm.tile([128, 1], f32, tag="mm2", name="mm2")
            nc.vector.tensor_mul(mm2[:], mean[:], mean[:])
            var = sm.tile([128, 1], f32, tag="var", name="var")
            nc.vector.scalar_tensor_tensor(
                out=var[:], in0=red[:, 1:2], scalar=1.0 / COUNT, in1=mm2[:],
                op0=ALU.mult, op1=ALU.subtract)
            rstd = sm.tile([128, 1], f32, tag="rstd", name="rstd")
            nc.scalar.activation(out=rstd[:], in_=var[:], func=AF.Sqrt, bias=epsb[:])
            nc.vector.reciprocal(rstd[:], rstd[:])
            nbias = sm.tile([128, 1], f32, tag="nbias", name="nbias")
            nc.vector.scalar_tensor_tensor(
                out=nbias[:], in0=mean[:], scalar=-1.0, in1=rstd[:],
                op0=ALU.mult, op1=ALU.mult)

            # ---- p = exp(rstd * scores + nbias), in place ----
            for ti in range(NT):
                nc.scalar.activation(out=st[ti][:], in_=st[ti][:], func=AF.Exp,
                                     bias=nbias[:], scale=rstd[:])

            # ---- softmax denominators (x H), reciprocal, broadcast ----
            den = sm.tile([1, S], f32, tag="den", name="den", bufs=1)
            for sci in range(NSC):
                sl = slice(sci * SCW, (sci + 1) * SCW)
                pd = pcs.tile([1, SCW], f32, tag="pd", name="pd")
                for ti in range(NT):
                    kk = 128 if ti < NT - 1 else PAD_REAL
                    nc.tensor.matmul(
                        pd[:], r(four[0:kk, :]), r(st[ti][0:kk, sl]),
                        start=(ti == 0), stop=(ti == NT - 1))
                nc.vector.tensor_copy(den[0:1, sl], pd[:])
            nc.vector.reciprocal(den[:], den[:])
            recipb = scr.tile([128, S], f32, tag="recipb", name="recipb", bufs=1)
            nc.gpsimd.partition_broadcast(recipb[:], den[0:1, :])

            # ---- ctx^T[d, s] = V^T p, scaled by 1/(H * denom) ----
            for dti in range(2):
                dsl = slice(dti * 128, (dti + 1) * 128)
                for sci in range(NSC):
                    sl = slice(sci * SCW, (sci + 1) * SCW)
                    ps = pcx.tile([128, SCW], f32, tag="psx", name="psx")
                    for ti in range(NT):
                        nc.tensor.matmul(ps[:], r(v[ti][:, dsl]), r(st[ti][:, sl]),
                                         start=(ti == 0), stop=(ti == NT - 1))
                    if h == 0:
                        nc.vector.tensor_mul(ctxs[dti][:, sl], ps[:], recipb[:, sl])
                    else:
                        t3 = scr.tile([128, SCW], f32, tag="t2", name="t3")
                        nc.vector.tensor_mul(t3[:], ps[:], recipb[:, sl])
                        nc.vector.tensor_add(ctxs[dti][:, sl], ctxs[dti][:, sl], t3[:])

        # ---- output projection: ot[e, s] = Wo^T @ (ctx0 + ctx1) ----
        for eti in range(2):
            esl = slice(eti * 128, (eti + 1) * 128)
            osb = scr.tile([128, S], f32, tag=f"osb{eti}", name=f"osb{eti}", bufs=1)
            for sci in range(NSC):
                sl = slice(sci * SCW, (sci + 1) * SCW)
                ps = pmm.tile([128, SCW], f32, tag="ps", name="ps")
                for cti in range(2):
                    nc.tensor.matmul(ps[:], r(wo[cti][:, esl]),
                                     r(ctxs[cti][:, sl]),
                                     start=(cti == 0), stop=(cti == 1))
                nc.vector.tensor_copy(osb[:, sl], ps[:])
            nc.sync.dma_start(ot_d[esl, :], osb[:])

    nc.finalize()
    return nc


def _get_nc(reps=1):
    key = ("nc", reps)
    if key not in _CACHE:
        _CACHE[key] = _build_nc(reps)
    return _CACHE[key]


def make_in_maps(emb, Wq, Wk, Wv, Wo):
    emb = np.ascontiguousarray(emb, dtype=np.float32)
    Wq = np.asarray(Wq, np.float64)
    Wk = np.asarray(Wk, np.float64)
    Wv = np.asarray(Wv, np.float64)
    Wo = np.asarray(Wo, np.float64)
    # wg[h] = Wq[h]^T @ Wk[h]  (scores = x wg^T x^T per head, see kernel docstring)
    wg = np.einsum("hdc,hde->hce", Wq, Wk).astype(np.float32)
    # wvo[h] = Wv[h]^T @ Wo^T  (folds the output projection into V)
    wvo = np.einsum("hdc,ed->hce", Wv, Wo).astype(np.float32)
    in_maps = []
    for core in range(8):
        b, g = core // 2, core % 2
        xt = np.zeros((C, SP), np.float32)
        xt[:, :S] = emb[b].transpose(1, 0, 2).reshape(C, S)
        hs = [2 * g, 2 * g + 1]
        in_maps.append({
            "xt": xt,
            "wg": np.ascontiguousarray(wg[hs]),
            "wvo": np.ascontiguousarray(wvo[hs]),
        })
    return in_maps


def gather_out(results):
    out = np.empty((B, S, C), np.float32)
    for b in range(B):
        out[b] = (results[2 * b]["ot"] + results[2 * b + 1]["ot"]).T
    return out.reshape(B, T, C, N)


def kernel(emb, Wq, Wk, Wv, Wo):
    from concourse.bass_utils import run_bass_kernel_spmd

    nc = _get_nc()
    in_maps = make_in_maps(emb, Wq, Wk, Wv, Wo)
    res = run_bass_kernel_spmd(nc, in_maps, list(range(8)))
    return gather_out(res.results)
